# revision 8
# baseline (speedup 1.0000x reference)
"""Trainium2 Bass kernel: batch-invariant causal multi-head attention (v4).

Sharding (8 NeuronCores): core c owns batch c//4 and head group c%4 (4 of 16
heads = 256 of 1024 features). Wq/Wk/Wv split column-wise by head group, Wo
row-wise; each core streams its batch's q/k/v pre-transposed to [E, S] f16.

Numerics: f16 activations x bf16 weights everywhere (fp8 fails the absmax
gate: ~3% quantization error on outlier attention rows amplifies to >2e-2).
Matmuls run 1 cyc/row (f16/bf16 rate).

Schedule (from TimelineSim iteration): exp on ACT (~60us busy) and PE
(~114us busy) dominate; the kernel is PE-bound, so every non-matmul op is
kept off the PE and the softmax pipeline keeps ACT/PE overlapped:
  - attention in 4 column passes of 512; within a pass, PV trails the
    scores/exp stream by one key-tile pair so PE never stalls on the
    current exp before issuing the next scores;
  - chunk j+1's projections and wo(j-1) ride as fillers inside chunk j's
    attention passes (consumed a pass later);
  - whole-chunk single DMAs (per-DMA dispatch is ~1.2us serialized);
  - input/weight DMAs on the SP/HWDGE queue in demand order;
  - exp(s/8) straight to f16 probability buffers (no max-shift: scores are
    O(8) and f16's range covers e^-inf..e^11; softmax shift-invariance);
  - softmax denominators from a ones-column in V; normalization via DVE
    reciprocal -> Pool partition_broadcast -> DVE multiply into f16 a;
  - Wo partials written bf16; host sums the 4 head-group partials per batch
    and adds Wo@bv + bo (V-bias folds out: attention rows sum to 1).
"""

import sys

if "/opt/trn_rl_repo" not in sys.path:
    sys.path.insert(0, "/opt/trn_rl_repo")

import numpy as np

S, B, E, H, D, P = 2048, 2, 1024, 16, 64, 128
NCORES = 8
CHUNK = 512               # projection chunk / pass width
NJ = S // CHUNK           # 4 chunks
NT = S // P               # 16 k-tiles
ET = E // P               # 8 e-tiles (contraction)
NEG = -1.0e9
PASSES = [(0, 512), (512, 512), (1024, 512), (1536, 512)]

_cache = {}


def _build_program():
    import concourse.tile as tile
    import concourse.mybir as mybir
    from concourse import bacc

    f32 = mybir.dt.float32
    f16 = mybir.dt.float16
    bf16 = mybir.dt.bfloat16
    AF = mybir.ActivationFunctionType

    nc = bacc.Bacc("TRN2", target_bir_lowering=False, debug=False)

    qt = nc.dram_tensor("qt", [E, S], f16, kind="ExternalInput").ap()
    kt = nc.dram_tensor("kt", [E, S], f16, kind="ExternalInput").ap()
    vt2 = nc.dram_tensor("vt2", [NJ, P, ET, 4, P], f16,
                         kind="ExternalInput").ap()
    wq = nc.dram_tensor("wq", [P, ET, 2, P], bf16, kind="ExternalInput").ap()
    wk = nc.dram_tensor("wk", [P, ET, 2, P], bf16, kind="ExternalInput").ap()
    wv = nc.dram_tensor("wv", [P, ET, 2 * P], bf16, kind="ExternalInput").ap()
    wo = nc.dram_tensor("wo", [P, ET, 2, P], bf16, kind="ExternalInput").ap()
    bqk = nc.dram_tensor("bqk", [P, 2, 2], f32, kind="ExternalInput").ap()
    maskblk = nc.dram_tensor("maskblk", [P, 2 * P], bf16, kind="ExternalInput").ap()
    identr = nc.dram_tensor("identr", [P, P], bf16, kind="ExternalInput").ap()
    outp = nc.dram_tensor("outp", [ET, P, S], bf16, kind="ExternalOutput").ap()

    escale = 1.0 / np.sqrt(D)

    with tile.TileContext(nc) as tc:
        with (
            tc.tile_pool(name="const", bufs=1) as cpool,
            tc.tile_pool(name="persist", bufs=1) as perst,
            tc.tile_pool(name="xin", bufs=3) as xin,
            tc.tile_pool(name="xv", bufs=2) as xvp,
            tc.tile_pool(name="ptile", bufs=4) as ppool,
            tc.tile_pool(name="recip", bufs=4) as rpool,
            tc.tile_pool(name="bcast", bufs=4) as bpool,
            tc.tile_pool(name="outsb", bufs=4) as opool,
            # PSUM budget (8 banks): mm 2x1 + s2 2x2 + pv 2x1 = 8
            tc.tile_pool(name="ps_mm", bufs=2, space="PSUM") as ps_mm,
            tc.tile_pool(name="ps_s", bufs=2, space="PSUM") as ps_s,
            tc.tile_pool(name="ps_pv", bufs=2, space="PSUM") as ps_pv,
        ):
            # ---- constants; demand-ordered DMAs on the serial DMA device
            wq_sb = cpool.tile([P, ET, 2, P], bf16, tag="wq")
            wk_sb = cpool.tile([P, ET, 2, P], bf16, tag="wk")
            wv_sb = cpool.tile([P, ET, 2 * P], bf16, tag="wv")
            wo_sb = cpool.tile([P, ET, 2, P], bf16, tag="wo")
            bqk_sb = cpool.tile([P, 2, 2], f32, tag="bqk")
            mask_sb = cpool.tile([P, 2 * P], bf16, tag="maskblk")
            idr_sb = cpool.tile([P, P], bf16, tag="identr")
            nc.sync.dma_start(bqk_sb[:], bqk[:])
            nc.sync.dma_start(wk_sb[:], wk[:])

            # warm the ACT exp table during DMA-bound startup
            warm = rpool.tile([P, 2], f32, tag="recip", name="warm",
                              padded_shape=[P, CHUNK])
            nc.scalar.activation(warm[:], bqk_sb[:, 0, :], AF.Exp, scale=1.0)

            # ---- persistent activations ----
            q_sb = perst.tile([P, 2, S], f16, tag="q")
            k_sb = perst.tile([P, 2, S], f16, tag="k")
            # attn output, pass-major [p, pass, plane, q]
            a_sb = perst.tile([P, NJ, 2, CHUNK], f16, tag="attnT")
            # V with ones column for the softmax denominators
            v_sb = perst.tile([P, NT, 4, 65], f16, tag="v")
            nc.vector.memset(v_sb[:, :, :, 64:65], 1.0)

            def proj_chunk(which, j):
                w_sb, x_ap, bi = {
                    "q": (wq_sb, qt, 0),
                    "k": (wk_sb, kt, 1),
                }[which]
                if which == "q" and j == 0:
                    nc.sync.dma_start(wq_sb[:], wq[:])
                cs = slice(CHUNK * j, CHUNK * (j + 1))
                ps0 = ps_mm.tile([P, CHUNK], f32, tag="mm")
                ps1 = ps_mm.tile([P, CHUNK], f32, tag="mm")
                xt = xin.tile([P, ET, CHUNK], f16, tag="xin")
                nc.sync.dma_start(
                    xt[:], x_ap[:, cs].rearrange("(o p) s -> p o s", p=P))
                for t in range(ET):
                    for dt, ps in ((0, ps0), (1, ps1)):
                        nc.tensor.matmul(
                            ps[:], w_sb[:, t, dt, :], xt[:, t, :],
                            start=(t == 0), stop=(t == ET - 1))
                for dt, ps in ((0, ps0), (1, ps1)):
                    dst = q_sb if which == "q" else k_sb
                    with nc.allow_low_precision(reason="f16 q/k"):
                        nc.vector.tensor_scalar_add(
                            dst[:, dt, cs], ps[:], bqk_sb[:, dt, bi:bi + 1])

            def proj_v_chunk(j):
                # x stationary, wv moving -> v in [s, feature] layout
                if j == 0:
                    nc.sync.dma_start(idr_sb[:], identr[:])
                    nc.sync.dma_start(mask_sb[:], maskblk[:])
                    nc.sync.dma_start(wv_sb[:], wv[:])
                xtv = xvp.tile([P, ET, 4, P], f16, tag="xv")
                nc.sync.dma_start(xtv[:], vt2[j])
                if j == 0:
                    nc.sync.dma_start(wo_sb[:], wo[:])
                for si in range(CHUNK // P):
                    kt_idx = (CHUNK // P) * j + si
                    psv = ps_mm.tile([P, 2 * P], f32, tag="mm", name="psv")
                    for t in range(ET):
                        nc.tensor.matmul(
                            psv[:], xtv[:, t, si, :], wv_sb[:, t, :],
                            start=(t == 0), stop=(t == ET - 1))
                    with nc.allow_low_precision(reason="f16 v"):
                        nc.vector.tensor_copy(
                            v_sb[:, kt_idx, :, 0:64],
                            psv[:].rearrange("p (h x) -> p h x", x=64))

            def attn_pass(dt, pp, fillers=()):
                # heads 2dt (partitions 0-63) and 2dt+1 (64-127).
                # fillers: thunks (next-chunk projections, trailing wo)
                # emitted one per pair behind the scores/exp stream so their
                # PE/DVE time hides under the ACT exp backlog.
                c0, w = PASSES[pp]
                t0 = c0 // P                     # first stairstep tile
                npair = (c0 + w) // (2 * P)
                pvo = [ps_pv.tile([65, CHUNK], f32, tag="pv", name=f"pv{_h}")
                       for _h in range(2)]

                def pv_emit(m, p_pair, rts):
                    for sl in range(2):
                        t = 2 * m + sl
                        r = rts[sl]
                        for hh in range(2):
                            hl = 2 * dt + hh
                            nc.tensor.matmul(
                                pvo[hh][:, r:w], v_sb[:, t, hl, :],
                                p_pair[:, hh, sl, r:w],
                                start=(t == 0), stop=(t == 2 * npair - 1))

                # software pipeline: PV one pair behind scores/exp
                prev = None
                for m in range(npair):
                    p_pair = ppool.tile([P, 2, 2, CHUNK], f16, tag="p")
                    rts = []
                    for sl in range(2):
                        t = 2 * m + sl
                        r = max(0, P * t - c0)
                        rts.append(r)
                        s2 = ps_s.tile([P, 2, CHUNK], f32, tag="s2", name="s2")
                        for hh in range(2):
                            hsl = slice(64 * hh, 64 * hh + 64)
                            nc.tensor.matmul(
                                s2[:, hh, r:w],
                                k_sb[hsl, dt, P * t:P * (t + 1)],
                                q_sb[hsl, dt, c0 + r:c0 + w],
                                start=True, stop=(t < t0))
                        if t >= t0:
                            nc.tensor.matmul(
                                s2[:, :, r:r + P], idr_sb[:], mask_sb[:],
                                start=False, stop=True)
                        nc.scalar.activation(
                            p_pair[:, :, sl, r:], s2[:, :, r:w], AF.Exp,
                            scale=escale)
                    if prev is not None:
                        pv_emit(*prev)
                    if m < len(fillers):
                        fillers[m]()
                    prev = (m, p_pair, rts)
                pv_emit(*prev)
                for f in fillers[npair:]:
                    f()
                for hh in range(2):
                    hs = slice(64 * hh, 64 * hh + 64)
                    rc = rpool.tile([1, CHUNK], f16, tag="recip")
                    bc = bpool.tile([64, CHUNK], f16, tag="bcast")
                    with nc.allow_low_precision(reason="f16 attn weights"):
                        nc.vector.reciprocal(rc[:], pvo[hh][64:65, :])
                        nc.gpsimd.partition_broadcast(bc[:], rc[:])
                        nc.vector.tensor_mul(
                            a_sb[hs, pp, dt, :], pvo[hh][0:64, :], bc[:])

            def wo_pass(pp):
                c0, w = PASSES[pp]
                last = pp == len(PASSES) - 1
                for tg in range(ET // 2):
                    ot = opool.tile([P, 2, w], bf16, tag="out")
                    for o in range(2):
                        t = 2 * tg + o
                        psum = ps_mm.tile([P, w], f32, tag="mm", name="wops")
                        for dt in range(2):
                            nc.tensor.matmul(psum[:], wo_sb[:, t, dt, :],
                                             a_sb[:, pp, dt, :],
                                             start=(dt == 0), stop=(dt == 1))
                        with nc.allow_low_precision(reason="bf16 partials"):
                            if last and o == 1:
                                # both engines idle at the tail: split copies
                                nc.scalar.activation(ot[:, o, :], psum[:],
                                                     AF.Copy)
                            else:
                                nc.vector.tensor_copy(ot[:, o, :], psum[:])
                    nc.sync.dma_start(
                        outp[2 * tg:2 * tg + 2, :, c0:c0 + w]
                        .rearrange("o p s -> p o s"), ot[:])

            def fk(j):
                return lambda: proj_chunk("k", j)

            def fq(j):
                return lambda: proj_chunk("q", j)

            def fv(j):
                return lambda: proj_v_chunk(j)

            def fw(p):
                return lambda: wo_pass(p)

            proj_chunk("k", 0)
            proj_chunk("q", 0)
            proj_v_chunk(0)
            for j in range(NJ - 1):
                attn_pass(0, j, (fk(j + 1), fq(j + 1)))
                attn_pass(1, j, (fv(j + 1),) + ((fw(j - 1),) if j else ()))
            attn_pass(0, NJ - 1, (fw(NJ - 2),))
            attn_pass(1, NJ - 1)
            wo_pass(NJ - 1)

    nc.compile()
    return nc


def _host_prep(query, key, value, Wq, bq, Wk, bk, Wv, bv, Wo, bo):
    import ml_dtypes
    f16 = np.float16
    bf16 = ml_dtypes.bfloat16

    qt = np.asarray(query, np.float32).transpose(1, 2, 0).astype(f16)
    kt = np.asarray(key, np.float32).transpose(1, 2, 0).astype(f16)
    vtT = np.asarray(value, np.float32).transpose(1, 2, 0).astype(f16)
    # vt2[j, p, t, si, f] = vT[128t+p, 512j+128si+f]
    vt2 = np.ascontiguousarray(
        vtT.reshape(B, ET, P, NJ, 4, P).transpose(0, 3, 2, 1, 4, 5))
    mb = np.where(np.arange(P)[None, :] >= np.arange(P)[:, None],
                  0.0, NEG).astype(bf16)
    maskblk = np.concatenate([mb, mb], axis=1)
    ident = np.eye(P, dtype=bf16)
    Wq, Wk, Wv, Wo = (np.asarray(a, np.float32) for a in (Wq, Wk, Wv, Wo))
    bq, bk = (np.asarray(a, np.float32) for a in (bq, bk))

    in_maps = []
    for c in range(NCORES):
        b, g = c // 4, c % 4
        F = slice(256 * g, 256 * (g + 1))
        in_maps.append({
            "qt": qt[b], "kt": kt[b], "vt2": vt2[b],
            "wq": np.ascontiguousarray(
                Wq[F, :].T.reshape(ET, P, 2, P).transpose(1, 0, 2, 3)
                .astype(bf16)),
            "wk": np.ascontiguousarray(
                Wk[F, :].T.reshape(ET, P, 2, P).transpose(1, 0, 2, 3)
                .astype(bf16)),
            "wv": np.ascontiguousarray(
                Wv[F, :].T.reshape(ET, P, 2 * P).transpose(1, 0, 2)
                .astype(bf16)),
            "wo": np.ascontiguousarray(
                Wo[:, F].T.reshape(2, P, ET, P).transpose(1, 2, 0, 3)
                .astype(bf16)),
            "bqk": np.ascontiguousarray(np.stack(
                [bq[F].reshape(2, P).T, bk[F].reshape(2, P).T], axis=2)),
            "maskblk": maskblk, "identr": ident,
        })
    return in_maps


def _get_runner():
    """Build the program once and wrap it in a jit-compiled 8-core SPMD
    executable that is reused across kernel() calls."""
    if "runner" in _cache:
        return _cache["runner"]

    import jax
    from jax.sharding import Mesh, PartitionSpec
    try:
        from jax.experimental.shard_map import shard_map
    except ImportError:
        from jax import shard_map
    import concourse.mybir as mybir
    import concourse.bass2jax as b2j

    nc = _cache.get("nc") or _build_program()
    _cache["nc"] = nc
    b2j.install_neuronx_cc_hook()

    in_names, out_names, out_avals, out_shapes = [], [], [], []
    for alloc in nc.m.functions[0].allocations:
        if not isinstance(alloc, mybir.MemoryLocationSet):
            continue
        name = alloc.memorylocations[0].name
        if alloc.kind == "ExternalInput":
            if nc.partition_id_tensor is None or name != nc.partition_id_tensor.name:
                in_names.append(name)
        elif alloc.kind == "ExternalOutput":
            out_names.append(name)
            shape = tuple(alloc.tensor_shape)
            dtype = mybir.dt.np(alloc.dtype)
            out_avals.append(jax.core.ShapedArray(shape, dtype))
            out_shapes.append((shape, dtype))
    n_params = len(in_names)
    all_in = list(in_names) + out_names
    pid_name = nc.partition_id_tensor.name if nc.partition_id_tensor else None
    if pid_name is not None:
        all_in.append(pid_name)

    def _body(*args):
        ops = list(args)
        if pid_name is not None:
            ops.append(b2j.partition_id_tensor())
        outs = b2j._bass_exec_p.bind(
            *ops, out_avals=tuple(out_avals), in_names=tuple(all_in),
            out_names=tuple(out_names), lowering_input_output_aliases=(),
            sim_require_finite=True, sim_require_nnan=True, nc=nc)
        return tuple(outs)

    devices = jax.devices()[:NCORES]
    mesh = Mesh(np.asarray(devices), ("core",))
    nio = n_params + len(out_names)
    sharded = jax.jit(
        shard_map(_body, mesh=mesh, in_specs=(PartitionSpec("core"),) * nio,
                  out_specs=(PartitionSpec("core"),) * len(out_names),
                  check_rep=False),
        donate_argnums=tuple(range(n_params, nio)), keep_unused=True)

    def run(in_maps):
        concat_in = [
            np.concatenate([np.asarray(in_maps[c][n]) for c in range(NCORES)], axis=0)
            for n in in_names]
        zeros = [np.zeros((NCORES * s[0], *s[1:]), d) for s, d in out_shapes]
        out_arrs = sharded(*concat_in, *zeros)
        return [
            {name: np.asarray(out_arrs[i]).reshape(NCORES, *out_shapes[i][0])[c]
             for i, name in enumerate(out_names)}
            for c in range(NCORES)]

    _cache["runner"] = run
    return run


def kernel(query, key, value, Wq, bq, Wk, bk, Wv, bv, Wo, bo):
    in_maps = _host_prep(query, key, value, Wq, bq, Wk, bk, Wv, bv, Wo, bo)

    results = None
    last_exc = None
    for attempt in range(3):
        try:
            results = _get_runner()(in_maps)
            break
        except Exception as exc:  # transient NRT/device wedges: rebuild + retry
            last_exc = exc
            _cache.pop("runner", None)
    if results is None:
        from concourse.bass_utils import run_bass_kernel_spmd
        nc = _cache.get("nc") or _build_program()
        _cache["nc"] = nc
        try:
            results = run_bass_kernel_spmd(
                nc, in_maps, core_ids=list(range(NCORES))).results
        except Exception:
            raise last_exc

    out = np.empty((S, B, E), np.float32)
    for b in range(B):
        acc = np.zeros((E, S), np.float32)
        for g in range(4):
            acc += results[4 * b + g]["outp"].reshape(E, S).astype(np.float32)
        out[:, b, :] = acc.T
    # attn rows sum to 1, so the V bias contributes the constant Wo @ bv
    const = (np.asarray(Wo, np.float64) @ np.asarray(bv, np.float64)
             + np.asarray(bo, np.float64)).astype(np.float32)
    return out + const


# revision 9
# speedup vs baseline: 1.0672x; 1.0672x over previous
"""Trainium2 Bass kernel: batch-invariant causal multi-head attention (v4).

Sharding (8 NeuronCores): core c owns batch c//4 and head group c%4 (4 of 16
heads = 256 of 1024 features). Wq/Wk/Wv split column-wise by head group, Wo
row-wise; each core streams its batch's q/k/v pre-transposed to [E, S] f16.

Numerics: f16 activations x bf16 weights everywhere (fp8 fails the absmax
gate: ~3% quantization error on outlier attention rows amplifies to >2e-2).
Matmuls run 1 cyc/row (f16/bf16 rate).

Schedule (from TimelineSim iteration): exp on ACT (~60us busy) and PE
(~114us busy) dominate; the kernel is PE-bound, so every non-matmul op is
kept off the PE and the softmax pipeline keeps ACT/PE overlapped:
  - attention in 4 column passes of 512; within a pass, PV trails the
    scores/exp stream by one key-tile pair so PE never stalls on the
    current exp before issuing the next scores;
  - chunk j+1's projections and wo(j-1) ride as fillers inside chunk j's
    attention passes (consumed a pass later);
  - whole-chunk single DMAs (per-DMA dispatch is ~1.2us serialized);
  - input/weight DMAs on the SP/HWDGE queue in demand order;
  - exp(s/8) straight to f16 probability buffers (no max-shift: scores are
    O(8) and f16's range covers e^-inf..e^11; softmax shift-invariance);
  - softmax denominators from a ones-column in V; normalization via DVE
    reciprocal -> Pool partition_broadcast -> DVE multiply into f16 a;
  - Wo partials written bf16; host sums the 4 head-group partials per batch
    and adds Wo@bv + bo (V-bias folds out: attention rows sum to 1).
"""

import sys

if "/opt/trn_rl_repo" not in sys.path:
    sys.path.insert(0, "/opt/trn_rl_repo")

import numpy as np

S, B, E, H, D, P = 2048, 2, 1024, 16, 64, 128
NCORES = 8
CHUNK = 512               # projection chunk / pass width
NJ = S // CHUNK           # 4 chunks
NT = S // P               # 16 k-tiles
EP = 4                    # e-tile DoubleRow pairs
WSC = 32.0                # fp8 weight scale
ET = E // P               # 8 e-tiles (contraction)
NEG = -1.0e9
PASSES = [(0, 512), (512, 512), (1024, 512), (1536, 512)]

_cache = {}


def _build_program():
    import concourse.tile as tile
    import concourse.mybir as mybir
    from concourse import bacc

    f32 = mybir.dt.float32
    f16 = mybir.dt.float16
    bf16 = mybir.dt.bfloat16
    AF = mybir.ActivationFunctionType

    nc = bacc.Bacc("TRN2", target_bir_lowering=False, debug=False)

    fp8 = mybir.dt.float8e4
    DRm = mybir.MatmulPerfMode.DoubleRow
    qth = nc.dram_tensor("qth", [E, S], fp8, kind="ExternalInput").ap()
    qtl = nc.dram_tensor("qtl", [E, S], fp8, kind="ExternalInput").ap()
    kth = nc.dram_tensor("kth", [E, S], fp8, kind="ExternalInput").ap()
    ktl = nc.dram_tensor("ktl", [E, S], fp8, kind="ExternalInput").ap()
    vt2h = nc.dram_tensor("vt2h", [NJ, P, EP, 4, 2, P], fp8,
                          kind="ExternalInput").ap()
    vt2l = nc.dram_tensor("vt2l", [NJ, P, EP, 4, 2, P], fp8,
                          kind="ExternalInput").ap()
    wqh = nc.dram_tensor("wqh", [P, EP, 2, 2, P], fp8, kind="ExternalInput").ap()
    wqr = nc.dram_tensor("wqr", [P, EP, 2, 2, P], fp8, kind="ExternalInput").ap()
    wkh = nc.dram_tensor("wkh", [P, EP, 2, 2, P], fp8, kind="ExternalInput").ap()
    wkr = nc.dram_tensor("wkr", [P, EP, 2, 2, P], fp8, kind="ExternalInput").ap()
    wvh = nc.dram_tensor("wvh", [P, EP, 2, 2 * P], fp8, kind="ExternalInput").ap()
    wvr = nc.dram_tensor("wvr", [P, EP, 2, 2 * P], fp8, kind="ExternalInput").ap()
    wo = nc.dram_tensor("wo", [P, ET, 2, P], bf16, kind="ExternalInput").ap()
    bqk = nc.dram_tensor("bqk", [P, 2, 2], f32, kind="ExternalInput").ap()
    maskblk = nc.dram_tensor("maskblk", [P, 2 * P], bf16, kind="ExternalInput").ap()
    identr = nc.dram_tensor("identr", [P, P], bf16, kind="ExternalInput").ap()
    outp = nc.dram_tensor("outp", [ET, P, S], bf16, kind="ExternalOutput").ap()

    escale = 1.0 / (WSC * WSC * np.sqrt(D))

    with tile.TileContext(nc) as tc:
        with (
            tc.tile_pool(name="const", bufs=1) as cpool,
            tc.tile_pool(name="persist", bufs=1) as perst,
            tc.tile_pool(name="xin", bufs=3) as xin,
            tc.tile_pool(name="xv", bufs=2) as xvp,
            tc.tile_pool(name="ptile", bufs=4) as ppool,
            tc.tile_pool(name="recip", bufs=4) as rpool,
            tc.tile_pool(name="bcast", bufs=4) as bpool,
            tc.tile_pool(name="outsb", bufs=4) as opool,
            # PSUM budget (8 banks): mm 2x1 + s2 2x2 + pv 2x1 = 8
            tc.tile_pool(name="ps_mm", bufs=2, space="PSUM") as ps_mm,
            tc.tile_pool(name="ps_s", bufs=2, space="PSUM") as ps_s,
            tc.tile_pool(name="ps_pv", bufs=2, space="PSUM") as ps_pv,
        ):
            # ---- constants; demand-ordered DMAs on the serial DMA device
            wq_sb = cpool.tile([P, EP, 2, 2, P], fp8, tag="wq")
            wqr_sb = cpool.tile([P, EP, 2, 2, P], fp8, tag="wqr")
            wk_sb = cpool.tile([P, EP, 2, 2, P], fp8, tag="wk")
            wkr_sb = cpool.tile([P, EP, 2, 2, P], fp8, tag="wkr")
            wv_sb = cpool.tile([P, EP, 2, 2 * P], fp8, tag="wv")
            wvr_sb = cpool.tile([P, EP, 2, 2 * P], fp8, tag="wvr")
            wo_sb = cpool.tile([P, ET, 2, P], bf16, tag="wo")
            bqk_sb = cpool.tile([P, 2, 2], f32, tag="bqk")
            mask_sb = cpool.tile([P, 2 * P], bf16, tag="maskblk")
            idr_sb = cpool.tile([P, P], bf16, tag="identr")
            nc.sync.dma_start(bqk_sb[:], bqk[:])
            nc.sync.dma_start(wk_sb[:], wkh[:])
            nc.sync.dma_start(wkr_sb[:], wkr[:])

            # warm the ACT exp table during DMA-bound startup
            warm = rpool.tile([P, 2], f32, tag="recip", name="warm",
                              padded_shape=[P, CHUNK])
            nc.scalar.activation(warm[:], bqk_sb[:, 0, :], AF.Exp, scale=1.0)

            # ---- persistent activations ----
            q_sb = perst.tile([P, 2, S], f16, tag="q")
            k_sb = perst.tile([P, 2, S], f16, tag="k")
            # attn output, pass-major [p, pass, plane, q]
            a_sb = perst.tile([P, NJ, 2, CHUNK], f16, tag="attnT")
            # V with ones column for the softmax denominators
            v_sb = perst.tile([P, NT, 4, 65], f16, tag="v")
            nc.vector.memset(v_sb[:, :, :, 64:65], 1.0)

            def proj_chunk(which, j):
                # 3-chain residual fp8 DoubleRow: xh@Wh + xh@Wr + xl@Wh
                # (xl@Wr ~0.1% dropped): 1.5x faster than f16/bf16 at
                # better-than-f16 accuracy; weights x32 into fp8, the x1024
                # on scores folds into the exp scale
                w_sb, wr_sb, xh_ap, xl_ap, bi = {
                    "q": (wq_sb, wqr_sb, qth, qtl, 0),
                    "k": (wk_sb, wkr_sb, kth, ktl, 1),
                }[which]
                if which == "q" and j == 0:
                    nc.sync.dma_start(wq_sb[:], wqh[:])
                    nc.sync.dma_start(wqr_sb[:], wqr[:])
                cs = slice(CHUNK * j, CHUNK * (j + 1))
                ps0 = ps_mm.tile([P, CHUNK], f32, tag="mm")
                ps1 = ps_mm.tile([P, CHUNK], f32, tag="mm")
                xh = xin.tile([P, ET, CHUNK], fp8, tag="xin")
                xl = xin.tile([P, ET, CHUNK], fp8, tag="xin")
                nc.sync.dma_start(
                    xh[:], xh_ap[:, cs].rearrange("(o p) s -> p o s", p=P))
                nc.sync.dma_start(
                    xl[:], xl_ap[:, cs].rearrange("(o p) s -> p o s", p=P))
                for g2 in range(EP):
                    for dt, ps in ((0, ps0), (1, ps1)):
                        for cw, cx in ((w_sb, xh), (wr_sb, xh), (w_sb, xl)):
                            nc.tensor.matmul(
                                ps[:], cw[:, g2, dt, :, :],
                                cx[:, 2 * g2:2 * g2 + 2, :],
                                start=(g2 == 0 and cw is w_sb and cx is xh),
                                stop=(g2 == EP - 1 and cx is xl),
                                perf_mode=DRm, skip_group_check=True)
                for dt, ps in ((0, ps0), (1, ps1)):
                    dst = q_sb if which == "q" else k_sb
                    with nc.allow_low_precision(reason="f16 q/k"):
                        nc.vector.tensor_scalar_add(
                            dst[:, dt, cs], ps[:], bqk_sb[:, dt, bi:bi + 1])

            def proj_v_chunk(j):
                # x stationary, wv moving -> v in [s, feature] layout
                if j == 0:
                    nc.sync.dma_start(idr_sb[:], identr[:])
                    nc.sync.dma_start(mask_sb[:], maskblk[:])
                    nc.sync.dma_start(wv_sb[:], wvh[:])
                    nc.sync.dma_start(wvr_sb[:], wvr[:])
                xtv = xvp.tile([P, EP, 4, 2, P], fp8, tag="xv")
                xtvl = xvp.tile([P, EP, 4, 2, P], fp8, tag="xv")
                nc.sync.dma_start(xtv[:], vt2h[j])
                nc.sync.dma_start(xtvl[:], vt2l[j])
                if j == 0:
                    nc.sync.dma_start(wo_sb[:], wo[:])
                for si in range(CHUNK // P):
                    kt_idx = (CHUNK // P) * j + si
                    psv = ps_mm.tile([P, 2 * P], f32, tag="mm", name="psv")
                    for g2 in range(EP):
                        for cx, cw in ((xtv, wv_sb), (xtvl, wv_sb),
                                       (xtv, wvr_sb)):
                            nc.tensor.matmul(
                                psv[:], cx[:, g2, si, :, :], cw[:, g2, :, :],
                                start=(g2 == 0 and cx is xtv and cw is wv_sb),
                                stop=(g2 == EP - 1 and cw is wvr_sb),
                                perf_mode=DRm, skip_group_check=True)
                    with nc.allow_low_precision(reason="f16 v"):
                        nc.vector.tensor_copy(
                            v_sb[:, kt_idx, :, 0:64],
                            psv[:].rearrange("p (h x) -> p h x", x=64))

            def attn_pass(dt, pp, fillers=()):
                # heads 2dt (partitions 0-63) and 2dt+1 (64-127).
                # fillers: thunks (next-chunk projections, trailing wo)
                # emitted one per pair behind the scores/exp stream so their
                # PE/DVE time hides under the ACT exp backlog.
                c0, w = PASSES[pp]
                t0 = c0 // P                     # first stairstep tile
                npair = (c0 + w) // (2 * P)
                pvo = [ps_pv.tile([65, CHUNK], f32, tag="pv", name=f"pv{_h}")
                       for _h in range(2)]

                def pv_emit(m, p_pair, rts):
                    for sl in range(2):
                        t = 2 * m + sl
                        r = rts[sl]
                        for hh in range(2):
                            hl = 2 * dt + hh
                            nc.tensor.matmul(
                                pvo[hh][:, r:w], v_sb[:, t, hl, :],
                                p_pair[:, hh, sl, r:w],
                                start=(t == 0), stop=(t == 2 * npair - 1))

                # software pipeline: PV two pairs behind scores/exp (PE is
                # the slower per-tile producer now; lag 1 left PE waiting on
                # the previous pair's exp at every PV)
                pend = []
                for m in range(npair):
                    p_pair = ppool.tile([P, 2, 2, CHUNK], f16, tag="p")
                    rts = []
                    for sl in range(2):
                        t = 2 * m + sl
                        r = max(0, P * t - c0)
                        rts.append(r)
                        s2 = ps_s.tile([P, 2, CHUNK], f32, tag="s2", name="s2")
                        for hh in range(2):
                            hsl = slice(64 * hh, 64 * hh + 64)
                            nc.tensor.matmul(
                                s2[:, hh, r:w],
                                k_sb[hsl, dt, P * t:P * (t + 1)],
                                q_sb[hsl, dt, c0 + r:c0 + w],
                                start=True, stop=(t < t0))
                        if t >= t0:
                            nc.tensor.matmul(
                                s2[:, :, r:r + P], idr_sb[:], mask_sb[:],
                                start=False, stop=True)
                        nc.scalar.activation(
                            p_pair[:, :, sl, r:], s2[:, :, r:w], AF.Exp,
                            scale=escale)
                    if len(pend) >= 2:
                        pv_emit(*pend.pop(0))
                    if m < len(fillers):
                        fillers[m]()
                    pend.append((m, p_pair, rts))
                for e in pend:
                    pv_emit(*e)
                for f in fillers[npair:]:
                    f()
                for hh in range(2):
                    hs = slice(64 * hh, 64 * hh + 64)
                    rc = rpool.tile([1, CHUNK], f16, tag="recip")
                    bc = bpool.tile([64, CHUNK], f16, tag="bcast")
                    with nc.allow_low_precision(reason="f16 attn weights"):
                        nc.vector.reciprocal(rc[:], pvo[hh][64:65, :])
                        nc.gpsimd.partition_broadcast(bc[:], rc[:])
                        nc.vector.tensor_mul(
                            a_sb[hs, pp, dt, :], pvo[hh][0:64, :], bc[:])

            def wo_pass(pp):
                c0, w = PASSES[pp]
                last = pp == len(PASSES) - 1
                for tg in range(ET // 2):
                    ot = opool.tile([P, 2, w], bf16, tag="out")
                    for o in range(2):
                        t = 2 * tg + o
                        psum = ps_mm.tile([P, w], f32, tag="mm", name="wops")
                        for dt in range(2):
                            nc.tensor.matmul(psum[:], wo_sb[:, t, dt, :],
                                             a_sb[:, pp, dt, :],
                                             start=(dt == 0), stop=(dt == 1))
                        with nc.allow_low_precision(reason="bf16 partials"):
                            if last and o == 1:
                                # both engines idle at the tail: split copies
                                nc.scalar.activation(ot[:, o, :], psum[:],
                                                     AF.Copy)
                            else:
                                nc.vector.tensor_copy(ot[:, o, :], psum[:])
                    nc.sync.dma_start(
                        outp[2 * tg:2 * tg + 2, :, c0:c0 + w]
                        .rearrange("o p s -> p o s"), ot[:])

            def fk(j):
                return lambda: proj_chunk("k", j)

            def fq(j):
                return lambda: proj_chunk("q", j)

            def fv(j):
                return lambda: proj_v_chunk(j)

            def fw(p):
                return lambda: wo_pass(p)

            proj_chunk("k", 0)
            proj_chunk("q", 0)
            proj_v_chunk(0)
            for j in range(NJ - 1):
                attn_pass(0, j, (fk(j + 1), fq(j + 1)))
                attn_pass(1, j, (fv(j + 1),) + ((fw(j - 1),) if j else ()))
            attn_pass(0, NJ - 1, (fw(NJ - 2),))
            attn_pass(1, NJ - 1)
            wo_pass(NJ - 1)

    nc.compile()
    return nc


def _host_prep(query, key, value, Wq, bq, Wk, bk, Wv, bv, Wo, bo):
    import ml_dtypes
    f16 = np.float16
    bf16 = ml_dtypes.bfloat16

    fp8 = ml_dtypes.float8_e4m3

    def hilo(a):
        h = a.astype(fp8)
        l = (a - h.astype(np.float32)).astype(fp8)
        return h, l

    qT = np.asarray(query, np.float32).transpose(1, 2, 0)
    kT = np.asarray(key, np.float32).transpose(1, 2, 0)
    vT = np.asarray(value, np.float32).transpose(1, 2, 0)
    qth, qtl = hilo(qT)
    kth, ktl = hilo(kT)
    vth, vtl = hilo(vT)

    def v2l(a):  # [B,E,S] -> [B, NJ, P, EP, 4, 2, P]
        return np.ascontiguousarray(
            a.reshape(B, EP, 2, P, NJ, 4, P).transpose(0, 4, 3, 1, 5, 2, 6))

    vt2h, vt2l_ = v2l(vth), v2l(vtl)
    mb = np.where(np.arange(P)[None, :] >= np.arange(P)[:, None],
                  0.0, NEG).astype(bf16)
    maskblk = np.concatenate([mb, mb], axis=1)
    ident = np.eye(P, dtype=bf16)
    Wq, Wk, Wv, Wo = (np.asarray(a, np.float32) for a in (Wq, Wk, Wv, Wo))
    bq, bk = (np.asarray(a, np.float32) * WSC for a in (bq, bk))

    def wqk_dr(W, F, part):
        WT = (W[F, :] * WSC).T                       # [E, 256]
        h = WT.astype(fp8)
        a = h if part == 0 else (WT - h.astype(np.float32)).astype(fp8)
        return np.ascontiguousarray(
            a.reshape(EP, 2, P, 2, P).transpose(2, 0, 3, 1, 4))

    def wv_dr(W, F, part):
        WT = (W[F, :] * WSC).T                       # [E, 256]
        h = WT.astype(fp8)
        a = h if part == 0 else (WT - h.astype(np.float32)).astype(fp8)
        return np.ascontiguousarray(
            a.reshape(EP, 2, P, 2 * P).transpose(2, 0, 1, 3))

    in_maps = []
    for c in range(NCORES):
        b, g = c // 4, c % 4
        F = slice(256 * g, 256 * (g + 1))
        in_maps.append({
            "qth": qth[b], "qtl": qtl[b], "kth": kth[b], "ktl": ktl[b],
            "vt2h": vt2h[b], "vt2l": vt2l_[b],
            "wqh": wqk_dr(Wq, F, 0), "wqr": wqk_dr(Wq, F, 1),
            "wkh": wqk_dr(Wk, F, 0), "wkr": wqk_dr(Wk, F, 1),
            "wvh": wv_dr(Wv, F, 0), "wvr": wv_dr(Wv, F, 1),
            "wo": np.ascontiguousarray(
                Wo[:, F].T.reshape(2, P, ET, P).transpose(1, 2, 0, 3)
                .astype(bf16)),
            "bqk": np.ascontiguousarray(np.stack(
                [bq[F].reshape(2, P).T, bk[F].reshape(2, P).T], axis=2)),
            "maskblk": maskblk, "identr": ident,
        })
    return in_maps


def _get_runner():
    """Build the program once and wrap it in a jit-compiled 8-core SPMD
    executable that is reused across kernel() calls."""
    if "runner" in _cache:
        return _cache["runner"]

    import jax
    from jax.sharding import Mesh, PartitionSpec
    try:
        from jax.experimental.shard_map import shard_map
    except ImportError:
        from jax import shard_map
    import concourse.mybir as mybir
    import concourse.bass2jax as b2j

    nc = _cache.get("nc") or _build_program()
    _cache["nc"] = nc
    b2j.install_neuronx_cc_hook()

    in_names, out_names, out_avals, out_shapes = [], [], [], []
    for alloc in nc.m.functions[0].allocations:
        if not isinstance(alloc, mybir.MemoryLocationSet):
            continue
        name = alloc.memorylocations[0].name
        if alloc.kind == "ExternalInput":
            if nc.partition_id_tensor is None or name != nc.partition_id_tensor.name:
                in_names.append(name)
        elif alloc.kind == "ExternalOutput":
            out_names.append(name)
            shape = tuple(alloc.tensor_shape)
            dtype = mybir.dt.np(alloc.dtype)
            out_avals.append(jax.core.ShapedArray(shape, dtype))
            out_shapes.append((shape, dtype))
    n_params = len(in_names)
    all_in = list(in_names) + out_names
    pid_name = nc.partition_id_tensor.name if nc.partition_id_tensor else None
    if pid_name is not None:
        all_in.append(pid_name)

    def _body(*args):
        ops = list(args)
        if pid_name is not None:
            ops.append(b2j.partition_id_tensor())
        outs = b2j._bass_exec_p.bind(
            *ops, out_avals=tuple(out_avals), in_names=tuple(all_in),
            out_names=tuple(out_names), lowering_input_output_aliases=(),
            sim_require_finite=True, sim_require_nnan=True, nc=nc)
        return tuple(outs)

    devices = jax.devices()[:NCORES]
    mesh = Mesh(np.asarray(devices), ("core",))
    nio = n_params + len(out_names)
    sharded = jax.jit(
        shard_map(_body, mesh=mesh, in_specs=(PartitionSpec("core"),) * nio,
                  out_specs=(PartitionSpec("core"),) * len(out_names),
                  check_rep=False),
        donate_argnums=tuple(range(n_params, nio)), keep_unused=True)

    def run(in_maps):
        concat_in = [
            np.concatenate([np.asarray(in_maps[c][n]) for c in range(NCORES)], axis=0)
            for n in in_names]
        zeros = [np.zeros((NCORES * s[0], *s[1:]), d) for s, d in out_shapes]
        out_arrs = sharded(*concat_in, *zeros)
        return [
            {name: np.asarray(out_arrs[i]).reshape(NCORES, *out_shapes[i][0])[c]
             for i, name in enumerate(out_names)}
            for c in range(NCORES)]

    _cache["runner"] = run
    return run


def kernel(query, key, value, Wq, bq, Wk, bk, Wv, bv, Wo, bo):
    in_maps = _host_prep(query, key, value, Wq, bq, Wk, bk, Wv, bv, Wo, bo)

    results = None
    last_exc = None
    for attempt in range(3):
        try:
            results = _get_runner()(in_maps)
            break
        except Exception as exc:  # transient NRT/device wedges: rebuild + retry
            last_exc = exc
            _cache.pop("runner", None)
    if results is None:
        from concourse.bass_utils import run_bass_kernel_spmd
        nc = _cache.get("nc") or _build_program()
        _cache["nc"] = nc
        try:
            results = run_bass_kernel_spmd(
                nc, in_maps, core_ids=list(range(NCORES))).results
        except Exception:
            raise last_exc

    out = np.empty((S, B, E), np.float32)
    for b in range(B):
        acc = np.zeros((E, S), np.float32)
        for g in range(4):
            acc += results[4 * b + g]["outp"].reshape(E, S).astype(np.float32)
        out[:, b, :] = acc.T / WSC
    # attn rows sum to 1, so the V bias contributes the constant Wo @ bv
    const = (np.asarray(Wo, np.float64) @ np.asarray(bv, np.float64)
             + np.asarray(bo, np.float64)).astype(np.float32)
    return out + const


# revision 10
# speedup vs baseline: 1.0921x; 1.0233x over previous
"""Trainium2 Bass kernel: batch-invariant causal multi-head attention (v4).

Sharding (8 NeuronCores): core c owns batch c//4 and head group c%4 (4 of 16
heads = 256 of 1024 features). Wq/Wk/Wv split column-wise by head group, Wo
row-wise; each core streams its batch's q/k/v pre-transposed to [E, S] f16.

Numerics: f16 activations x bf16 weights everywhere (fp8 fails the absmax
gate: ~3% quantization error on outlier attention rows amplifies to >2e-2).
Matmuls run 1 cyc/row (f16/bf16 rate).

Schedule (from TimelineSim iteration): exp on ACT (~60us busy) and PE
(~114us busy) dominate; the kernel is PE-bound, so every non-matmul op is
kept off the PE and the softmax pipeline keeps ACT/PE overlapped:
  - attention in 4 column passes of 512; within a pass, PV trails the
    scores/exp stream by one key-tile pair so PE never stalls on the
    current exp before issuing the next scores;
  - chunk j+1's projections and wo(j-1) ride as fillers inside chunk j's
    attention passes (consumed a pass later);
  - whole-chunk single DMAs (per-DMA dispatch is ~1.2us serialized);
  - input/weight DMAs on the SP/HWDGE queue in demand order;
  - exp(s/8) straight to f16 probability buffers (no max-shift: scores are
    O(8) and f16's range covers e^-inf..e^11; softmax shift-invariance);
  - softmax denominators from a ones-column in V; normalization via DVE
    reciprocal -> Pool partition_broadcast -> DVE multiply into f16 a;
  - Wo partials written bf16; host sums the 4 head-group partials per batch
    and adds Wo@bv + bo (V-bias folds out: attention rows sum to 1).
"""

import sys

if "/opt/trn_rl_repo" not in sys.path:
    sys.path.insert(0, "/opt/trn_rl_repo")

import numpy as np

S, B, E, H, D, P = 2048, 2, 1024, 16, 64, 128
NCORES = 8
CHUNK = 512               # projection chunk / pass width
NJ = S // CHUNK           # 4 chunks
NT = S // P               # 16 k-tiles
EP = 4                    # e-tile DoubleRow pairs
WSC = 32.0                # fp8 weight scale
ET = E // P               # 8 e-tiles (contraction)
NEG = -1.0e9
PASSES = [(0, 512), (512, 512), (1024, 512), (1536, 512)]

_cache = {}


def _build_program():
    import concourse.tile as tile
    import concourse.mybir as mybir
    from concourse import bacc

    f32 = mybir.dt.float32
    f16 = mybir.dt.float16
    bf16 = mybir.dt.bfloat16
    AF = mybir.ActivationFunctionType

    nc = bacc.Bacc("TRN2", target_bir_lowering=False, debug=False)

    fp8 = mybir.dt.float8e4
    DRm = mybir.MatmulPerfMode.DoubleRow
    qth = nc.dram_tensor("qth", [E, S], fp8, kind="ExternalInput").ap()
    qtl = nc.dram_tensor("qtl", [E, S], fp8, kind="ExternalInput").ap()
    kth = nc.dram_tensor("kth", [E, S], fp8, kind="ExternalInput").ap()
    ktl = nc.dram_tensor("ktl", [E, S], fp8, kind="ExternalInput").ap()
    vt2h = nc.dram_tensor("vt2h", [NJ, P, EP, 4, 2, P], fp8,
                          kind="ExternalInput").ap()
    vt2l = nc.dram_tensor("vt2l", [NJ, P, EP, 4, 2, P], fp8,
                          kind="ExternalInput").ap()
    wqh = nc.dram_tensor("wqh", [P, EP, 2, 2, P], fp8, kind="ExternalInput").ap()
    wqr = nc.dram_tensor("wqr", [P, EP, 2, 2, P], fp8, kind="ExternalInput").ap()
    wkh = nc.dram_tensor("wkh", [P, EP, 2, 2, P], fp8, kind="ExternalInput").ap()
    wkr = nc.dram_tensor("wkr", [P, EP, 2, 2, P], fp8, kind="ExternalInput").ap()
    wvh = nc.dram_tensor("wvh", [P, EP, 2, 2 * P], fp8, kind="ExternalInput").ap()
    wvr = nc.dram_tensor("wvr", [P, EP, 2, 2 * P], fp8, kind="ExternalInput").ap()
    wo = nc.dram_tensor("wo", [P, ET, 2, P], bf16, kind="ExternalInput").ap()
    bqk = nc.dram_tensor("bqk", [P, 2, 2], f32, kind="ExternalInput").ap()
    maskblk = nc.dram_tensor("maskblk", [P, 2 * P], bf16, kind="ExternalInput").ap()
    identr = nc.dram_tensor("identr", [P, P], bf16, kind="ExternalInput").ap()
    outp = nc.dram_tensor("outp", [ET, P, S], bf16, kind="ExternalOutput").ap()

    escale = 1.0 / (WSC * WSC * np.sqrt(D))

    with tile.TileContext(nc) as tc:
        with (
            tc.tile_pool(name="const", bufs=1) as cpool,
            tc.tile_pool(name="persist", bufs=1) as perst,
            tc.tile_pool(name="xin", bufs=5) as xin,
            tc.tile_pool(name="xv", bufs=4) as xvp,
            tc.tile_pool(name="ptile", bufs=4) as ppool,
            tc.tile_pool(name="recip", bufs=4) as rpool,
            tc.tile_pool(name="bcast", bufs=4) as bpool,
            tc.tile_pool(name="outsb", bufs=4) as opool,
            # PSUM budget (8 banks): mm 2x1 + s2 2x2 + pv 2x1 = 8
            tc.tile_pool(name="ps_mm", bufs=2, space="PSUM") as ps_mm,
            tc.tile_pool(name="ps_s", bufs=2, space="PSUM") as ps_s,
            tc.tile_pool(name="ps_pv", bufs=2, space="PSUM") as ps_pv,
        ):
            # ---- constants; demand-ordered DMAs on the serial DMA device
            wq_sb = cpool.tile([P, EP, 2, 2, P], fp8, tag="wq")
            wqr_sb = cpool.tile([P, EP, 2, 2, P], fp8, tag="wqr")
            wk_sb = cpool.tile([P, EP, 2, 2, P], fp8, tag="wk")
            wkr_sb = cpool.tile([P, EP, 2, 2, P], fp8, tag="wkr")
            wv_sb = cpool.tile([P, EP, 2, 2 * P], fp8, tag="wv")
            wvr_sb = cpool.tile([P, EP, 2, 2 * P], fp8, tag="wvr")
            wo_sb = cpool.tile([P, ET, 2, P], bf16, tag="wo")
            bqk_sb = cpool.tile([P, 2, 2], f32, tag="bqk")
            mask_sb = cpool.tile([P, 2 * P], bf16, tag="maskblk")
            idr_sb = cpool.tile([P, P], bf16, tag="identr")
            nc.sync.dma_start(bqk_sb[:], bqk[:])
            nc.sync.dma_start(wk_sb[:], wkh[:])
            nc.sync.dma_start(wkr_sb[:], wkr[:])

            # warm the ACT exp table during DMA-bound startup
            warm = rpool.tile([P, 2], f32, tag="recip", name="warm",
                              padded_shape=[P, CHUNK])
            nc.scalar.activation(warm[:], bqk_sb[:, 0, :], AF.Exp, scale=1.0)

            # ---- persistent activations ----
            q_sb = perst.tile([P, 2, S], f16, tag="q")
            k_sb = perst.tile([P, 2, S], f16, tag="k")
            # attn output, pass-major [p, pass, plane, q]
            a_sb = perst.tile([P, NJ, 2, CHUNK], f16, tag="attnT")
            # V with ones column for the softmax denominators
            v_sb = perst.tile([P, NT, 4, 65], f16, tag="v")
            nc.vector.memset(v_sb[:, :, :, 64:65], 1.0)

            def proj_chunk(which, j):
                # 3-chain residual fp8 DoubleRow: xh@Wh + xh@Wr + xl@Wh
                # (xl@Wr ~0.1% dropped): 1.5x faster than f16/bf16 at
                # better-than-f16 accuracy; weights x32 into fp8, the x1024
                # on scores folds into the exp scale
                w_sb, wr_sb, xh_ap, xl_ap, bi = {
                    "q": (wq_sb, wqr_sb, qth, qtl, 0),
                    "k": (wk_sb, wkr_sb, kth, ktl, 1),
                }[which]
                if which == "q" and j == 0:
                    nc.sync.dma_start(wq_sb[:], wqh[:])
                    nc.sync.dma_start(wqr_sb[:], wqr[:])
                cs = slice(CHUNK * j, CHUNK * (j + 1))
                ps0 = ps_mm.tile([P, CHUNK], f32, tag="mm")
                ps1 = ps_mm.tile([P, CHUNK], f32, tag="mm")
                xh = xin.tile([P, ET, CHUNK], fp8, tag="xin")
                xl = xin.tile([P, ET, CHUNK], fp8, tag="xin")
                nc.sync.dma_start(
                    xh[:], xh_ap[:, cs].rearrange("(o p) s -> p o s", p=P))
                nc.sync.dma_start(
                    xl[:], xl_ap[:, cs].rearrange("(o p) s -> p o s", p=P))
                for g2 in range(EP):
                    for dt, ps in ((0, ps0), (1, ps1)):
                        for cw, cx in ((w_sb, xh), (wr_sb, xh), (w_sb, xl)):
                            nc.tensor.matmul(
                                ps[:], cw[:, g2, dt, :, :],
                                cx[:, 2 * g2:2 * g2 + 2, :],
                                start=(g2 == 0 and cw is w_sb and cx is xh),
                                stop=(g2 == EP - 1 and cx is xl),
                                perf_mode=DRm, skip_group_check=True)
                for dt, ps in ((0, ps0), (1, ps1)):
                    dst = q_sb if which == "q" else k_sb
                    with nc.allow_low_precision(reason="f16 q/k"):
                        nc.vector.tensor_scalar_add(
                            dst[:, dt, cs], ps[:], bqk_sb[:, dt, bi:bi + 1])

            def proj_v_chunk(j):
                # x stationary, wv moving -> v in [s, feature] layout
                if j == 0:
                    nc.sync.dma_start(idr_sb[:], identr[:])
                    nc.sync.dma_start(mask_sb[:], maskblk[:])
                    nc.sync.dma_start(wv_sb[:], wvh[:])
                    nc.sync.dma_start(wvr_sb[:], wvr[:])
                xtv = xvp.tile([P, EP, 4, 2, P], fp8, tag="xv")
                xtvl = xvp.tile([P, EP, 4, 2, P], fp8, tag="xv")
                nc.sync.dma_start(xtv[:], vt2h[j])
                nc.sync.dma_start(xtvl[:], vt2l[j])
                if j == 0:
                    nc.sync.dma_start(wo_sb[:], wo[:])
                for si in range(CHUNK // P):
                    kt_idx = (CHUNK // P) * j + si
                    psv = ps_mm.tile([P, 2 * P], f32, tag="mm", name="psv")
                    for g2 in range(EP):
                        for cx, cw in ((xtv, wv_sb), (xtvl, wv_sb),
                                       (xtv, wvr_sb)):
                            nc.tensor.matmul(
                                psv[:], cx[:, g2, si, :, :], cw[:, g2, :, :],
                                start=(g2 == 0 and cx is xtv and cw is wv_sb),
                                stop=(g2 == EP - 1 and cw is wvr_sb),
                                perf_mode=DRm, skip_group_check=True)
                    with nc.allow_low_precision(reason="f16 v"):
                        nc.vector.tensor_copy(
                            v_sb[:, kt_idx, :, 0:64],
                            psv[:].rearrange("p (h x) -> p h x", x=64))

            def attn_pass(dt, pp, fillers=()):
                # heads 2dt (partitions 0-63) and 2dt+1 (64-127).
                # fillers: thunks (next-chunk projections, trailing wo)
                # emitted one per pair behind the scores/exp stream so their
                # PE/DVE time hides under the ACT exp backlog.
                c0, w = PASSES[pp]
                t0 = c0 // P                     # first stairstep tile
                npair = (c0 + w) // (2 * P)
                pvo = [ps_pv.tile([65, CHUNK], f32, tag="pv", name=f"pv{_h}")
                       for _h in range(2)]

                def pv_emit(m, p_pair, rts):
                    for sl in range(2):
                        t = 2 * m + sl
                        r = rts[sl]
                        for hh in range(2):
                            hl = 2 * dt + hh
                            nc.tensor.matmul(
                                pvo[hh][:, r:w], v_sb[:, t, hl, :],
                                p_pair[:, hh, sl, r:w],
                                start=(t == 0), stop=(t == 2 * npair - 1))

                # software pipeline: PV two pairs behind scores/exp (PE is
                # the slower per-tile producer now; lag 1 left PE waiting on
                # the previous pair's exp at every PV)
                pend = []
                for m in range(npair):
                    p_pair = ppool.tile([P, 2, 2, CHUNK], f16, tag="p")
                    rts = []
                    for sl in range(2):
                        t = 2 * m + sl
                        r = max(0, P * t - c0)
                        rts.append(r)
                        s2 = ps_s.tile([P, 2, CHUNK], f32, tag="s2", name="s2")
                        for hh in range(2):
                            hsl = slice(64 * hh, 64 * hh + 64)
                            nc.tensor.matmul(
                                s2[:, hh, r:w],
                                k_sb[hsl, dt, P * t:P * (t + 1)],
                                q_sb[hsl, dt, c0 + r:c0 + w],
                                start=True, stop=(t < t0))
                        if t >= t0:
                            nc.tensor.matmul(
                                s2[:, :, r:r + P], idr_sb[:], mask_sb[:],
                                start=False, stop=True)
                        nc.scalar.activation(
                            p_pair[:, :, sl, r:], s2[:, :, r:w], AF.Exp,
                            scale=escale)
                    if len(pend) >= 2:
                        pv_emit(*pend.pop(0))
                    if m < len(fillers):
                        fillers[m]()
                    pend.append((m, p_pair, rts))
                for e in pend:
                    pv_emit(*e)
                for f in fillers[npair:]:
                    f()
                for hh in range(2):
                    hs = slice(64 * hh, 64 * hh + 64)
                    rc = rpool.tile([1, CHUNK], f16, tag="recip")
                    bc = bpool.tile([64, CHUNK], f16, tag="bcast")
                    with nc.allow_low_precision(reason="f16 attn weights"):
                        nc.vector.reciprocal(rc[:], pvo[hh][64:65, :])
                        nc.gpsimd.partition_broadcast(bc[:], rc[:])
                        nc.vector.tensor_mul(
                            a_sb[hs, pp, dt, :], pvo[hh][0:64, :], bc[:])

            def wo_pass(pp):
                c0, w = PASSES[pp]
                last = pp == len(PASSES) - 1
                for tg in range(ET // 2):
                    ot = opool.tile([P, 2, w], bf16, tag="out")
                    for o in range(2):
                        t = 2 * tg + o
                        psum = ps_mm.tile([P, w], f32, tag="mm", name="wops")
                        for dt in range(2):
                            nc.tensor.matmul(psum[:], wo_sb[:, t, dt, :],
                                             a_sb[:, pp, dt, :],
                                             start=(dt == 0), stop=(dt == 1))
                        with nc.allow_low_precision(reason="bf16 partials"):
                            if last and o == 1:
                                # both engines idle at the tail: split copies
                                nc.scalar.activation(ot[:, o, :], psum[:],
                                                     AF.Copy)
                            else:
                                nc.vector.tensor_copy(ot[:, o, :], psum[:])
                    nc.sync.dma_start(
                        outp[2 * tg:2 * tg + 2, :, c0:c0 + w]
                        .rearrange("o p s -> p o s"), ot[:])

            def fk(j):
                return lambda: proj_chunk("k", j)

            def fq(j):
                return lambda: proj_chunk("q", j)

            def fv(j):
                return lambda: proj_v_chunk(j)

            def fw(p):
                return lambda: wo_pass(p)

            proj_chunk("k", 0)
            proj_chunk("q", 0)
            proj_v_chunk(0)
            for j in range(NJ - 1):
                attn_pass(0, j, (fk(j + 1), fq(j + 1)))
                attn_pass(1, j, (fv(j + 1),) + ((fw(j - 1),) if j else ()))
            attn_pass(0, NJ - 1, (fw(NJ - 2),))
            attn_pass(1, NJ - 1)
            wo_pass(NJ - 1)

    nc.compile()
    return nc


def _host_prep(query, key, value, Wq, bq, Wk, bk, Wv, bv, Wo, bo):
    import ml_dtypes
    f16 = np.float16
    bf16 = ml_dtypes.bfloat16

    fp8 = ml_dtypes.float8_e4m3

    def hilo(a):
        h = a.astype(fp8)
        l = (a - h.astype(np.float32)).astype(fp8)
        return h, l

    qT = np.asarray(query, np.float32).transpose(1, 2, 0)
    kT = np.asarray(key, np.float32).transpose(1, 2, 0)
    vT = np.asarray(value, np.float32).transpose(1, 2, 0)
    qth, qtl = hilo(qT)
    kth, ktl = hilo(kT)
    vth, vtl = hilo(vT)

    def v2l(a):  # [B,E,S] -> [B, NJ, P, EP, 4, 2, P]
        return np.ascontiguousarray(
            a.reshape(B, EP, 2, P, NJ, 4, P).transpose(0, 4, 3, 1, 5, 2, 6))

    vt2h, vt2l_ = v2l(vth), v2l(vtl)
    mb = np.where(np.arange(P)[None, :] >= np.arange(P)[:, None],
                  0.0, NEG).astype(bf16)
    maskblk = np.concatenate([mb, mb], axis=1)
    ident = np.eye(P, dtype=bf16)
    Wq, Wk, Wv, Wo = (np.asarray(a, np.float32) for a in (Wq, Wk, Wv, Wo))
    bq, bk = (np.asarray(a, np.float32) * WSC for a in (bq, bk))

    def wqk_dr(W, F, part):
        WT = (W[F, :] * WSC).T                       # [E, 256]
        h = WT.astype(fp8)
        a = h if part == 0 else (WT - h.astype(np.float32)).astype(fp8)
        return np.ascontiguousarray(
            a.reshape(EP, 2, P, 2, P).transpose(2, 0, 3, 1, 4))

    def wv_dr(W, F, part):
        WT = (W[F, :] * WSC).T                       # [E, 256]
        h = WT.astype(fp8)
        a = h if part == 0 else (WT - h.astype(np.float32)).astype(fp8)
        return np.ascontiguousarray(
            a.reshape(EP, 2, P, 2 * P).transpose(2, 0, 1, 3))

    in_maps = []
    for c in range(NCORES):
        b, g = c // 4, c % 4
        F = slice(256 * g, 256 * (g + 1))
        in_maps.append({
            "qth": qth[b], "qtl": qtl[b], "kth": kth[b], "ktl": ktl[b],
            "vt2h": vt2h[b], "vt2l": vt2l_[b],
            "wqh": wqk_dr(Wq, F, 0), "wqr": wqk_dr(Wq, F, 1),
            "wkh": wqk_dr(Wk, F, 0), "wkr": wqk_dr(Wk, F, 1),
            "wvh": wv_dr(Wv, F, 0), "wvr": wv_dr(Wv, F, 1),
            "wo": np.ascontiguousarray(
                Wo[:, F].T.reshape(2, P, ET, P).transpose(1, 2, 0, 3)
                .astype(bf16)),
            "bqk": np.ascontiguousarray(np.stack(
                [bq[F].reshape(2, P).T, bk[F].reshape(2, P).T], axis=2)),
            "maskblk": maskblk, "identr": ident,
        })
    return in_maps


def _get_runner():
    """Build the program once and wrap it in a jit-compiled 8-core SPMD
    executable that is reused across kernel() calls."""
    if "runner" in _cache:
        return _cache["runner"]

    import jax
    from jax.sharding import Mesh, PartitionSpec
    try:
        from jax.experimental.shard_map import shard_map
    except ImportError:
        from jax import shard_map
    import concourse.mybir as mybir
    import concourse.bass2jax as b2j

    nc = _cache.get("nc") or _build_program()
    _cache["nc"] = nc
    b2j.install_neuronx_cc_hook()

    in_names, out_names, out_avals, out_shapes = [], [], [], []
    for alloc in nc.m.functions[0].allocations:
        if not isinstance(alloc, mybir.MemoryLocationSet):
            continue
        name = alloc.memorylocations[0].name
        if alloc.kind == "ExternalInput":
            if nc.partition_id_tensor is None or name != nc.partition_id_tensor.name:
                in_names.append(name)
        elif alloc.kind == "ExternalOutput":
            out_names.append(name)
            shape = tuple(alloc.tensor_shape)
            dtype = mybir.dt.np(alloc.dtype)
            out_avals.append(jax.core.ShapedArray(shape, dtype))
            out_shapes.append((shape, dtype))
    n_params = len(in_names)
    all_in = list(in_names) + out_names
    pid_name = nc.partition_id_tensor.name if nc.partition_id_tensor else None
    if pid_name is not None:
        all_in.append(pid_name)

    def _body(*args):
        ops = list(args)
        if pid_name is not None:
            ops.append(b2j.partition_id_tensor())
        outs = b2j._bass_exec_p.bind(
            *ops, out_avals=tuple(out_avals), in_names=tuple(all_in),
            out_names=tuple(out_names), lowering_input_output_aliases=(),
            sim_require_finite=True, sim_require_nnan=True, nc=nc)
        return tuple(outs)

    devices = jax.devices()[:NCORES]
    mesh = Mesh(np.asarray(devices), ("core",))
    nio = n_params + len(out_names)
    sharded = jax.jit(
        shard_map(_body, mesh=mesh, in_specs=(PartitionSpec("core"),) * nio,
                  out_specs=(PartitionSpec("core"),) * len(out_names),
                  check_rep=False),
        donate_argnums=tuple(range(n_params, nio)), keep_unused=True)

    def run(in_maps):
        concat_in = [
            np.concatenate([np.asarray(in_maps[c][n]) for c in range(NCORES)], axis=0)
            for n in in_names]
        zeros = [np.zeros((NCORES * s[0], *s[1:]), d) for s, d in out_shapes]
        out_arrs = sharded(*concat_in, *zeros)
        return [
            {name: np.asarray(out_arrs[i]).reshape(NCORES, *out_shapes[i][0])[c]
             for i, name in enumerate(out_names)}
            for c in range(NCORES)]

    _cache["runner"] = run
    return run


def kernel(query, key, value, Wq, bq, Wk, bk, Wv, bv, Wo, bo):
    in_maps = _host_prep(query, key, value, Wq, bq, Wk, bk, Wv, bv, Wo, bo)

    results = None
    last_exc = None
    for attempt in range(3):
        try:
            results = _get_runner()(in_maps)
            break
        except Exception as exc:  # transient NRT/device wedges: rebuild + retry
            last_exc = exc
            _cache.pop("runner", None)
    if results is None:
        from concourse.bass_utils import run_bass_kernel_spmd
        nc = _cache.get("nc") or _build_program()
        _cache["nc"] = nc
        try:
            results = run_bass_kernel_spmd(
                nc, in_maps, core_ids=list(range(NCORES))).results
        except Exception:
            raise last_exc

    out = np.empty((S, B, E), np.float32)
    for b in range(B):
        acc = np.zeros((E, S), np.float32)
        for g in range(4):
            acc += results[4 * b + g]["outp"].reshape(E, S).astype(np.float32)
        out[:, b, :] = acc.T / WSC
    # attn rows sum to 1, so the V bias contributes the constant Wo @ bv
    const = (np.asarray(Wo, np.float64) @ np.asarray(bv, np.float64)
             + np.asarray(bo, np.float64)).astype(np.float32)
    return out + const


# revision 11
# speedup vs baseline: 1.0925x; 1.0004x over previous
"""Trainium2 Bass kernel: batch-invariant causal multi-head attention (v4).

Sharding (8 NeuronCores): core c owns batch c//4 and head group c%4 (4 of 16
heads = 256 of 1024 features). Wq/Wk/Wv split column-wise by head group, Wo
row-wise; each core streams its batch's q/k/v pre-transposed to [E, S] f16.

Numerics: f16 activations x bf16 weights everywhere (fp8 fails the absmax
gate: ~3% quantization error on outlier attention rows amplifies to >2e-2).
Matmuls run 1 cyc/row (f16/bf16 rate).

Schedule (from TimelineSim iteration): exp on ACT (~60us busy) and PE
(~114us busy) dominate; the kernel is PE-bound, so every non-matmul op is
kept off the PE and the softmax pipeline keeps ACT/PE overlapped:
  - attention in 4 column passes of 512; within a pass, PV trails the
    scores/exp stream by one key-tile pair so PE never stalls on the
    current exp before issuing the next scores;
  - chunk j+1's projections and wo(j-1) ride as fillers inside chunk j's
    attention passes (consumed a pass later);
  - whole-chunk single DMAs (per-DMA dispatch is ~1.2us serialized);
  - input/weight DMAs on the SP/HWDGE queue in demand order;
  - exp(s/8) straight to f16 probability buffers (no max-shift: scores are
    O(8) and f16's range covers e^-inf..e^11; softmax shift-invariance);
  - softmax denominators from a ones-column in V; normalization via DVE
    reciprocal -> Pool partition_broadcast -> DVE multiply into f16 a;
  - Wo partials written bf16; host sums the 4 head-group partials per batch
    and adds Wo@bv + bo (V-bias folds out: attention rows sum to 1).
"""

import sys

if "/opt/trn_rl_repo" not in sys.path:
    sys.path.insert(0, "/opt/trn_rl_repo")

import numpy as np

S, B, E, H, D, P = 2048, 2, 1024, 16, 64, 128
NCORES = 8
CHUNK = 512               # projection chunk / pass width
NJ = S // CHUNK           # 4 chunks
NT = S // P               # 16 k-tiles
EP = 4                    # e-tile DoubleRow pairs
WSC = 32.0                # fp8 weight scale
ET = E // P               # 8 e-tiles (contraction)
NEG = -1.0e9
PASSES = [(0, 512), (512, 512), (1024, 512), (1536, 512)]

_cache = {}


def _build_program():
    import concourse.tile as tile
    import concourse.mybir as mybir
    from concourse import bacc

    f32 = mybir.dt.float32
    f16 = mybir.dt.float16
    bf16 = mybir.dt.bfloat16
    AF = mybir.ActivationFunctionType

    nc = bacc.Bacc("TRN2", target_bir_lowering=False, debug=False)

    fp8 = mybir.dt.float8e4
    DRm = mybir.MatmulPerfMode.DoubleRow
    qth = nc.dram_tensor("qth", [E, S], fp8, kind="ExternalInput").ap()
    qtl = nc.dram_tensor("qtl", [E, S], fp8, kind="ExternalInput").ap()
    kth = nc.dram_tensor("kth", [E, S], fp8, kind="ExternalInput").ap()
    ktl = nc.dram_tensor("ktl", [E, S], fp8, kind="ExternalInput").ap()
    vt2h = nc.dram_tensor("vt2h", [NJ, P, EP, 4, 2, P], fp8,
                          kind="ExternalInput").ap()
    vt2l = nc.dram_tensor("vt2l", [NJ, P, EP, 4, 2, P], fp8,
                          kind="ExternalInput").ap()
    wqh = nc.dram_tensor("wqh", [P, EP, 2, 2, P], fp8, kind="ExternalInput").ap()
    wqr = nc.dram_tensor("wqr", [P, EP, 2, 2, P], fp8, kind="ExternalInput").ap()
    wkh = nc.dram_tensor("wkh", [P, EP, 2, 2, P], fp8, kind="ExternalInput").ap()
    wkr = nc.dram_tensor("wkr", [P, EP, 2, 2, P], fp8, kind="ExternalInput").ap()
    wvh = nc.dram_tensor("wvh", [P, EP, 2, 2 * P], fp8, kind="ExternalInput").ap()
    wvr = nc.dram_tensor("wvr", [P, EP, 2, 2 * P], fp8, kind="ExternalInput").ap()
    wo = nc.dram_tensor("wo", [P, ET, 2, P], bf16, kind="ExternalInput").ap()
    bqk = nc.dram_tensor("bqk", [P, 2, 2], f32, kind="ExternalInput").ap()
    maskblk = nc.dram_tensor("maskblk", [P, 2 * P], bf16, kind="ExternalInput").ap()
    identr = nc.dram_tensor("identr", [P, P], bf16, kind="ExternalInput").ap()
    outp = nc.dram_tensor("outp", [ET, P, S], bf16, kind="ExternalOutput").ap()

    escale = 1.0 / (WSC * WSC * np.sqrt(D))

    with tile.TileContext(nc) as tc:
        with (
            tc.tile_pool(name="const", bufs=1) as cpool,
            tc.tile_pool(name="persist", bufs=1) as perst,
            tc.tile_pool(name="xin", bufs=5) as xin,
            tc.tile_pool(name="xv", bufs=4) as xvp,
            tc.tile_pool(name="ptile", bufs=6) as ppool,
            tc.tile_pool(name="recip", bufs=4) as rpool,
            tc.tile_pool(name="bcast", bufs=4) as bpool,
            tc.tile_pool(name="outsb", bufs=6) as opool,
            # PSUM budget (8 banks): mm 2x1 + s2 2x2 + pv 2x1 = 8
            tc.tile_pool(name="ps_mm", bufs=2, space="PSUM") as ps_mm,
            tc.tile_pool(name="ps_s", bufs=2, space="PSUM") as ps_s,
            tc.tile_pool(name="ps_pv", bufs=2, space="PSUM") as ps_pv,
        ):
            # ---- constants; demand-ordered DMAs on the serial DMA device
            wq_sb = cpool.tile([P, EP, 2, 2, P], fp8, tag="wq")
            wqr_sb = cpool.tile([P, EP, 2, 2, P], fp8, tag="wqr")
            wk_sb = cpool.tile([P, EP, 2, 2, P], fp8, tag="wk")
            wkr_sb = cpool.tile([P, EP, 2, 2, P], fp8, tag="wkr")
            wv_sb = cpool.tile([P, EP, 2, 2 * P], fp8, tag="wv")
            wvr_sb = cpool.tile([P, EP, 2, 2 * P], fp8, tag="wvr")
            wo_sb = cpool.tile([P, ET, 2, P], bf16, tag="wo")
            bqk_sb = cpool.tile([P, 2, 2], f32, tag="bqk")
            mask_sb = cpool.tile([P, 2 * P], bf16, tag="maskblk")
            idr_sb = cpool.tile([P, P], bf16, tag="identr")
            nc.sync.dma_start(bqk_sb[:], bqk[:])
            nc.sync.dma_start(wk_sb[:], wkh[:])

            # warm the ACT exp table during DMA-bound startup
            warm = rpool.tile([P, 2], f32, tag="recip", name="warm",
                              padded_shape=[P, CHUNK])
            nc.scalar.activation(warm[:], bqk_sb[:, 0, :], AF.Exp, scale=1.0)

            # ---- persistent activations ----
            q_sb = perst.tile([P, 2, S], f16, tag="q")
            k_sb = perst.tile([P, 2, S], f16, tag="k")
            # attn output, pass-major [p, pass, plane, q]
            a_sb = perst.tile([P, NJ, 2, CHUNK], f16, tag="attnT")
            # V with ones column for the softmax denominators
            v_sb = perst.tile([P, NT, 4, 65], f16, tag="v")
            nc.vector.memset(v_sb[:, :, :, 64:65], 1.0)

            def proj_dma(which, j):
                xh_ap, xl_ap = (qth, qtl) if which == "q" else (kth, ktl)
                if which == "q" and j == 0:
                    nc.sync.dma_start(wq_sb[:], wqh[:])
                cs = slice(CHUNK * j, CHUNK * (j + 1))
                xh = xin.tile([P, ET, CHUNK], fp8, tag="xin")
                xl = xin.tile([P, ET, CHUNK], fp8, tag="xin")
                nc.sync.dma_start(
                    xh[:], xh_ap[:, cs].rearrange("(o p) s -> p o s", p=P))
                nc.sync.dma_start(
                    xl[:], xl_ap[:, cs].rearrange("(o p) s -> p o s", p=P))
                if j == 0:
                    wr_dram = {"q": wqr_sb[:], "k": wkr_sb[:]}
                    wr_src = {"q": wqr[:], "k": wkr[:]}
                    nc.sync.dma_start(wr_dram[which], wr_src[which])
                return xh, xl

            def proj_chunk(which, j, pre=None):
                # 3-chain residual fp8 DoubleRow: xh@Wh + xh@Wr + xl@Wh
                # (xl@Wr ~0.1% dropped): 1.5x faster than f16/bf16 at
                # better-than-f16 accuracy; weights x32 into fp8, the x1024
                # on scores folds into the exp scale
                w_sb, wr_sb, xh_ap, xl_ap, bi = {
                    "q": (wq_sb, wqr_sb, qth, qtl, 0),
                    "k": (wk_sb, wkr_sb, kth, ktl, 1),
                }[which]
                wr_dram = {"q": wqr_sb[:], "k": wkr_sb[:]}
                wr_src = {"q": wqr[:], "k": wkr[:]}
                if which == "q" and j == 0:
                    nc.sync.dma_start(wq_sb[:], wqh[:])
                cs = slice(CHUNK * j, CHUNK * (j + 1))
                if pre is None:
                    xh = xin.tile([P, ET, CHUNK], fp8, tag="xin")
                    xl = xin.tile([P, ET, CHUNK], fp8, tag="xin")
                    nc.sync.dma_start(
                        xh[:], xh_ap[:, cs].rearrange("(o p) s -> p o s", p=P))
                    nc.sync.dma_start(
                        xl[:], xl_ap[:, cs].rearrange("(o p) s -> p o s", p=P))
                    if j == 0:
                        nc.sync.dma_start(wr_dram[which], wr_src[which])
                else:
                    xh, xl = pre
                ps0 = ps_mm.tile([P, CHUNK], f32, tag="mm")
                ps1 = ps_mm.tile([P, CHUNK], f32, tag="mm")
                # hi-chain first: the residual-weight DMA overlaps it
                for cw, cx in ((w_sb, xh), (wr_sb, xh), (w_sb, xl)):
                    for g2 in range(EP):
                        for dt, ps in ((0, ps0), (1, ps1)):
                            nc.tensor.matmul(
                                ps[:], cw[:, g2, dt, :, :],
                                cx[:, 2 * g2:2 * g2 + 2, :],
                                start=(g2 == 0 and cw is w_sb and cx is xh),
                                stop=(g2 == EP - 1 and cx is xl),
                                perf_mode=DRm, skip_group_check=True)
                for dt, ps in ((0, ps0), (1, ps1)):
                    dst = q_sb if which == "q" else k_sb
                    with nc.allow_low_precision(reason="f16 q/k"):
                        nc.vector.tensor_scalar_add(
                            dst[:, dt, cs], ps[:], bqk_sb[:, dt, bi:bi + 1])

            def proj_v_dma(j):
                xtv = xvp.tile([P, EP, 4, 2, P], fp8, tag="xv")
                xtvl = xvp.tile([P, EP, 4, 2, P], fp8, tag="xv")
                nc.sync.dma_start(xtv[:], vt2h[j])
                nc.sync.dma_start(xtvl[:], vt2l[j])
                if j == 0:
                    nc.sync.dma_start(wo_sb[:], wo[:])
                return xtv, xtvl

            def proj_v_chunk(j, pre=None):
                # x stationary, wv moving -> v in [s, feature] layout
                if j == 0:
                    nc.sync.dma_start(idr_sb[:], identr[:])
                    nc.sync.dma_start(mask_sb[:], maskblk[:])
                    nc.sync.dma_start(wv_sb[:], wvh[:])
                    nc.sync.dma_start(wvr_sb[:], wvr[:])
                if pre is None:
                    pre = proj_v_dma(j)
                xtv, xtvl = pre
                for si in range(CHUNK // P):
                    kt_idx = (CHUNK // P) * j + si
                    psv = ps_mm.tile([P, 2 * P], f32, tag="mm", name="psv")
                    for g2 in range(EP):
                        for cx, cw in ((xtv, wv_sb), (xtvl, wv_sb),
                                       (xtv, wvr_sb)):
                            nc.tensor.matmul(
                                psv[:], cx[:, g2, si, :, :], cw[:, g2, :, :],
                                start=(g2 == 0 and cx is xtv and cw is wv_sb),
                                stop=(g2 == EP - 1 and cw is wvr_sb),
                                perf_mode=DRm, skip_group_check=True)
                    with nc.allow_low_precision(reason="f16 v"):
                        nc.vector.tensor_copy(
                            v_sb[:, kt_idx, :, 0:64],
                            psv[:].rearrange("p (h x) -> p h x", x=64))

            def attn_pass(dt, pp, head=(), tail=()):
                # heads 2dt (partitions 0-63) and 2dt+1 (64-127).
                # fillers: thunks (next-chunk projections, trailing wo)
                # emitted one per pair behind the scores/exp stream so their
                # PE/DVE time hides under the ACT exp backlog.
                c0, w = PASSES[pp]
                t0 = c0 // P                     # first stairstep tile
                npair = (c0 + w) // (2 * P)
                pvo = [ps_pv.tile([65, CHUNK], f32, tag="pv", name=f"pv{_h}")
                       for _h in range(2)]

                def pv_emit(m, p_pair, rts):
                    for sl in range(2):
                        t = 2 * m + sl
                        r = rts[sl]
                        for hh in range(2):
                            hl = 2 * dt + hh
                            nc.tensor.matmul(
                                pvo[hh][:, r:w], v_sb[:, t, hl, :],
                                p_pair[:, hh, sl, r:w],
                                start=(t == 0), stop=(t == 2 * npair - 1))

                # software pipeline: PV two pairs behind scores/exp (PE is
                # the slower per-tile producer now; lag 1 left PE waiting on
                # the previous pair's exp at every PV)
                pend = []
                for m in range(npair):
                    p_pair = ppool.tile([P, 2, 2, CHUNK], f16, tag="p")
                    rts = []
                    for sl in range(2):
                        t = 2 * m + sl
                        r = max(0, P * t - c0)
                        rts.append(r)
                        s2 = ps_s.tile([P, 2, CHUNK], f32, tag="s2", name="s2")
                        for hh in range(2):
                            hsl = slice(64 * hh, 64 * hh + 64)
                            nc.tensor.matmul(
                                s2[:, hh, r:w],
                                k_sb[hsl, dt, P * t:P * (t + 1)],
                                q_sb[hsl, dt, c0 + r:c0 + w],
                                start=True, stop=(t < t0))
                        if t >= t0:
                            nc.tensor.matmul(
                                s2[:, :, r:r + P], idr_sb[:], mask_sb[:],
                                start=False, stop=True)
                        nc.scalar.activation(
                            p_pair[:, :, sl, r:], s2[:, :, r:w], AF.Exp,
                            scale=escale)
                    if len(pend) >= 2:
                        pv_emit(*pend.pop(0))
                    # head fillers (DMA issues) early; tail fillers (the
                    # corresponding matmuls) at the LAST pairs, where PE
                    # otherwise idles waiting the pass-end exps
                    if m < len(head):
                        head[m]()
                    fidx = m - (npair - len(tail))
                    if fidx >= 0:
                        tail[fidx]()
                    pend.append((m, p_pair, rts))
                for e in pend:
                    pv_emit(*e)
                for hh in range(2):
                    hs = slice(64 * hh, 64 * hh + 64)
                    rc = rpool.tile([1, CHUNK], f16, tag="recip")
                    bc = bpool.tile([64, CHUNK], f16, tag="bcast")
                    with nc.allow_low_precision(reason="f16 attn weights"):
                        nc.vector.reciprocal(rc[:], pvo[hh][64:65, :])
                        nc.gpsimd.partition_broadcast(bc[:], rc[:])
                        nc.vector.tensor_mul(
                            a_sb[hs, pp, dt, :], pvo[hh][0:64, :], bc[:])

            def wo_pass(pp):
                c0, w = PASSES[pp]
                last = pp == len(PASSES) - 1
                for tg in range(ET // 2):
                    ot = opool.tile([P, 2, w], bf16, tag="out")
                    for o in range(2):
                        t = 2 * tg + o
                        psum = ps_mm.tile([P, w], f32, tag="mm", name="wops")
                        for dt in range(2):
                            nc.tensor.matmul(psum[:], wo_sb[:, t, dt, :],
                                             a_sb[:, pp, dt, :],
                                             start=(dt == 0), stop=(dt == 1))
                        with nc.allow_low_precision(reason="bf16 partials"):
                            if last and o == 1:
                                # both engines idle at the tail: split copies
                                nc.scalar.activation(ot[:, o, :], psum[:],
                                                     AF.Copy)
                            else:
                                nc.vector.tensor_copy(ot[:, o, :], psum[:])
                    nc.sync.dma_start(
                        outp[2 * tg:2 * tg + 2, :, c0:c0 + w]
                        .rearrange("o p s -> p o s"), ot[:])

            def split(fn_dma, fn_mm):
                box = {}

                def d():
                    box["t"] = fn_dma()

                def m():
                    fn_mm(box["t"])
                return d, m

            def fw(p):
                return lambda: wo_pass(p)

            proj_chunk("k", 0)
            proj_chunk("q", 0)
            proj_v_chunk(0)
            for j in range(NJ - 1):
                kd, km = split(lambda w="k", jj=j + 1: proj_dma(w, jj),
                               lambda t, w="k", jj=j + 1: proj_chunk(w, jj, t))
                qd, qm = split(lambda w="q", jj=j + 1: proj_dma(w, jj),
                               lambda t, w="q", jj=j + 1: proj_chunk(w, jj, t))
                vd, vm = split(lambda jj=j + 1: proj_v_dma(jj),
                               lambda t, jj=j + 1: proj_v_chunk(jj, t))
                attn_pass(0, j, head=(kd, qd), tail=(km, qm))
                attn_pass(1, j, head=(vd,),
                          tail=(vm,) + ((fw(j - 1),) if j else ()))
            attn_pass(0, NJ - 1, tail=(fw(NJ - 2),))
            attn_pass(1, NJ - 1)
            wo_pass(NJ - 1)

    nc.compile()
    return nc


def _host_prep(query, key, value, Wq, bq, Wk, bk, Wv, bv, Wo, bo):
    import ml_dtypes
    f16 = np.float16
    bf16 = ml_dtypes.bfloat16

    fp8 = ml_dtypes.float8_e4m3

    def hilo(a):
        h = a.astype(fp8)
        l = (a - h.astype(np.float32)).astype(fp8)
        return h, l

    qT = np.asarray(query, np.float32).transpose(1, 2, 0)
    kT = np.asarray(key, np.float32).transpose(1, 2, 0)
    vT = np.asarray(value, np.float32).transpose(1, 2, 0)
    qth, qtl = hilo(qT)
    kth, ktl = hilo(kT)
    vth, vtl = hilo(vT)

    def v2l(a):  # [B,E,S] -> [B, NJ, P, EP, 4, 2, P]
        return np.ascontiguousarray(
            a.reshape(B, EP, 2, P, NJ, 4, P).transpose(0, 4, 3, 1, 5, 2, 6))

    vt2h, vt2l_ = v2l(vth), v2l(vtl)
    mb = np.where(np.arange(P)[None, :] >= np.arange(P)[:, None],
                  0.0, NEG).astype(bf16)
    maskblk = np.concatenate([mb, mb], axis=1)
    ident = np.eye(P, dtype=bf16)
    Wq, Wk, Wv, Wo = (np.asarray(a, np.float32) for a in (Wq, Wk, Wv, Wo))
    bq, bk = (np.asarray(a, np.float32) * WSC for a in (bq, bk))

    def wqk_dr(W, F, part):
        WT = (W[F, :] * WSC).T                       # [E, 256]
        h = WT.astype(fp8)
        a = h if part == 0 else (WT - h.astype(np.float32)).astype(fp8)
        return np.ascontiguousarray(
            a.reshape(EP, 2, P, 2, P).transpose(2, 0, 3, 1, 4))

    def wv_dr(W, F, part):
        WT = (W[F, :] * WSC).T                       # [E, 256]
        h = WT.astype(fp8)
        a = h if part == 0 else (WT - h.astype(np.float32)).astype(fp8)
        return np.ascontiguousarray(
            a.reshape(EP, 2, P, 2 * P).transpose(2, 0, 1, 3))

    in_maps = []
    for c in range(NCORES):
        b, g = c // 4, c % 4
        F = slice(256 * g, 256 * (g + 1))
        in_maps.append({
            "qth": qth[b], "qtl": qtl[b], "kth": kth[b], "ktl": ktl[b],
            "vt2h": vt2h[b], "vt2l": vt2l_[b],
            "wqh": wqk_dr(Wq, F, 0), "wqr": wqk_dr(Wq, F, 1),
            "wkh": wqk_dr(Wk, F, 0), "wkr": wqk_dr(Wk, F, 1),
            "wvh": wv_dr(Wv, F, 0), "wvr": wv_dr(Wv, F, 1),
            "wo": np.ascontiguousarray(
                Wo[:, F].T.reshape(2, P, ET, P).transpose(1, 2, 0, 3)
                .astype(bf16)),
            "bqk": np.ascontiguousarray(np.stack(
                [bq[F].reshape(2, P).T, bk[F].reshape(2, P).T], axis=2)),
            "maskblk": maskblk, "identr": ident,
        })
    return in_maps


def _get_runner():
    """Build the program once and wrap it in a jit-compiled 8-core SPMD
    executable that is reused across kernel() calls."""
    if "runner" in _cache:
        return _cache["runner"]

    import jax
    from jax.sharding import Mesh, PartitionSpec
    try:
        from jax.experimental.shard_map import shard_map
    except ImportError:
        from jax import shard_map
    import concourse.mybir as mybir
    import concourse.bass2jax as b2j

    nc = _cache.get("nc") or _build_program()
    _cache["nc"] = nc
    b2j.install_neuronx_cc_hook()

    in_names, out_names, out_avals, out_shapes = [], [], [], []
    for alloc in nc.m.functions[0].allocations:
        if not isinstance(alloc, mybir.MemoryLocationSet):
            continue
        name = alloc.memorylocations[0].name
        if alloc.kind == "ExternalInput":
            if nc.partition_id_tensor is None or name != nc.partition_id_tensor.name:
                in_names.append(name)
        elif alloc.kind == "ExternalOutput":
            out_names.append(name)
            shape = tuple(alloc.tensor_shape)
            dtype = mybir.dt.np(alloc.dtype)
            out_avals.append(jax.core.ShapedArray(shape, dtype))
            out_shapes.append((shape, dtype))
    n_params = len(in_names)
    all_in = list(in_names) + out_names
    pid_name = nc.partition_id_tensor.name if nc.partition_id_tensor else None
    if pid_name is not None:
        all_in.append(pid_name)

    def _body(*args):
        ops = list(args)
        if pid_name is not None:
            ops.append(b2j.partition_id_tensor())
        outs = b2j._bass_exec_p.bind(
            *ops, out_avals=tuple(out_avals), in_names=tuple(all_in),
            out_names=tuple(out_names), lowering_input_output_aliases=(),
            sim_require_finite=True, sim_require_nnan=True, nc=nc)
        return tuple(outs)

    devices = jax.devices()[:NCORES]
    mesh = Mesh(np.asarray(devices), ("core",))
    nio = n_params + len(out_names)
    sharded = jax.jit(
        shard_map(_body, mesh=mesh, in_specs=(PartitionSpec("core"),) * nio,
                  out_specs=(PartitionSpec("core"),) * len(out_names),
                  check_rep=False),
        donate_argnums=tuple(range(n_params, nio)), keep_unused=True)

    def run(in_maps):
        concat_in = [
            np.concatenate([np.asarray(in_maps[c][n]) for c in range(NCORES)], axis=0)
            for n in in_names]
        zeros = [np.zeros((NCORES * s[0], *s[1:]), d) for s, d in out_shapes]
        out_arrs = sharded(*concat_in, *zeros)
        return [
            {name: np.asarray(out_arrs[i]).reshape(NCORES, *out_shapes[i][0])[c]
             for i, name in enumerate(out_names)}
            for c in range(NCORES)]

    _cache["runner"] = run
    return run


def kernel(query, key, value, Wq, bq, Wk, bk, Wv, bv, Wo, bo):
    in_maps = _host_prep(query, key, value, Wq, bq, Wk, bk, Wv, bv, Wo, bo)

    results = None
    last_exc = None
    for attempt in range(3):
        try:
            results = _get_runner()(in_maps)
            break
        except Exception as exc:  # transient NRT/device wedges: rebuild + retry
            last_exc = exc
            _cache.pop("runner", None)
    if results is None:
        from concourse.bass_utils import run_bass_kernel_spmd
        nc = _cache.get("nc") or _build_program()
        _cache["nc"] = nc
        try:
            results = run_bass_kernel_spmd(
                nc, in_maps, core_ids=list(range(NCORES))).results
        except Exception:
            raise last_exc

    out = np.empty((S, B, E), np.float32)
    for b in range(B):
        acc = np.zeros((E, S), np.float32)
        for g in range(4):
            acc += results[4 * b + g]["outp"].reshape(E, S).astype(np.float32)
        out[:, b, :] = acc.T / WSC
    # attn rows sum to 1, so the V bias contributes the constant Wo @ bv
    const = (np.asarray(Wo, np.float64) @ np.asarray(bv, np.float64)
             + np.asarray(bo, np.float64)).astype(np.float32)
    return out + const
